# revision 2
# baseline (speedup 1.0000x reference)
"""Trainium2 Bass kernel for DequantingLinear (GGML Q8_0 dequant + linear).

Computes out[4096, 12288] = x[4096, 3072] @ dequant(w_q, w_scales).T + bias
where w_q is int32 (int8-valued) with per-32-element-block fp32 scales.

Sharding: tensor-parallel over output features across 8 NeuronCores. Each
core gets the full x and a 1536-row shard of w_q / w_scales / bias,
computes its [4096, 1536] output slice; the host concatenates on axis 1.

Per-core kernel (Tile framework):
  * w path: per 128-row o-tile, load w_q int32 -> SBUF, one mixed-dtype
    vector multiply (int32 x block-broadcast fp32 scales -> bf16, exact for
    |q|<=127), then 24 SBUF->SBUF xbar-transpose DMAs of [128,128] chunks
    directly into the resident [in, k, out] SBUF weight tensor. No DRAM
    bounce for weights.
  * x path: fp32 -> bf16 DRAM->DRAM SWDGE casts into a 4-slot ring of
    internal DRAM tensors, then per-block xbar-transpose loads to
    [in, k, tok]. Casts beyond the first two are paced by a dependency on
    the xt loads two blocks back so bulk x traffic cannot starve the
    w chain or the out writes.
  * GEMM: psum[128 tok, 512 out] tiles accumulate 24 bf16 k-tile matmuls
    (fp32 PSUM); bias is added during the PSUM->SBUF copy on the vector
    engine. Phase-1 GEMMs (first two token blocks, n=0 columns) are issued
    as soon as the first 4 o-tiles of w land, filling the pipeline head.
  All HWDGE DMAs are issued on nc.sync - ACT-issued DMAs were observed to
  produce corrupted results on hardware in this configuration.
"""

import sys

for _p in ("/opt/trn_rl_repo",):
    if _p not in sys.path:
        sys.path.append(_p)

from contextlib import ExitStack

import numpy as np

import concourse.bacc as bacc
import concourse.bass as bass
import concourse.mybir as mybir
from concourse import tile
from concourse.tile_rust import add_dep_helper
from concourse.bass_utils import run_bass_kernel_spmd

FP32 = mybir.dt.float32
BF16 = mybir.dt.bfloat16
INT32 = mybir.dt.int32

N_CORES = 8
TOK, IN, OUT = 4096, 3072, 12288
QK = 32
OUT_SH = OUT // N_CORES
TOK_BLK = 512
NCOL = 512
RING_SLOTS = 4
FREE_CASTS = 2
NB1 = 2


def build_kernel(nc: bass.Bass, repeats: int = 1):
    P = 128
    KT = IN // P          # 24 k-tiles
    NBLK = TOK // TOK_BLK  # 8 token blocks
    MT = TOK_BLK // P     # 4 m-tiles per block
    NT = OUT_SH // NCOL   # 3 n-column groups
    NB = IN // QK         # 96 scale blocks
    OT = OUT_SH // P      # 12 o-tiles
    OT_PER_N = NCOL // P  # 4 o-tiles per n-group

    x = nc.dram_tensor("x", [TOK, IN], FP32, kind="ExternalInput")
    w_q = nc.dram_tensor("w_q", [OUT_SH, IN], INT32, kind="ExternalInput")
    w_scales = nc.dram_tensor("w_scales", [OUT_SH, NB], FP32, kind="ExternalInput")
    bias = nc.dram_tensor("bias", [OUT_SH], FP32, kind="ExternalInput")
    out = nc.dram_tensor("out", [TOK, OUT_SH], FP32, kind="ExternalOutput")

    x_slots = [
        nc.dram_tensor(f"x_bf16_{s}", [TOK_BLK, IN], BF16) for s in range(RING_SLOTS)
    ]

    with tile.TileContext(nc) as tc, ExitStack() as ctx:
        const_pool = ctx.enter_context(tc.tile_pool(name="const", bufs=1))
        wq_pool = ctx.enter_context(tc.tile_pool(name="wq", bufs=3))
        wd_pool = ctx.enter_context(tc.tile_pool(name="wd", bufs=2))
        wt_pool = ctx.enter_context(tc.tile_pool(name="wt", bufs=1))
        xt_pool = ctx.enter_context(tc.tile_pool(name="xt", bufs=2))
        out_pool = ctx.enter_context(tc.tile_pool(name="out", bufs=4))
        psum_pool = ctx.enter_context(tc.tile_pool(name="psum", bufs=8, space="PSUM"))

        for _rep in range(repeats):
            bias_rep = const_pool.tile([P, OUT_SH], FP32, tag="bias_rep")
            nc.sync.dma_start(
                bias_rep[:], bias.ap().unsqueeze(0).to_broadcast([P, OUT_SH])
            )

            sc_tiles = []
            for o in range(OT):
                sct = const_pool.tile([P, NB], FP32, tag=f"sc_{o}")
                nc.sync.dma_start(sct[:], w_scales.ap()[o * P : (o + 1) * P, :])
                sc_tiles.append(sct)

            def cast_block(b):
                s = b % RING_SLOTS
                srows = slice(b * TOK_BLK, (b + 1) * TOK_BLK)
                return nc.gpsimd.dma_start(x_slots[s].ap()[:, :], x.ap()[srows, :])

            for b in range(min(FREE_CASTS, NBLK)):
                cast_block(b)

            def load_xt(b):
                s = b % RING_SLOTS
                xt = xt_pool.tile([P, KT, TOK_BLK], BF16, tag="xt")
                last = None
                for k in range(KT):
                    last = nc.sync.dma_start(
                        xt[:, k, :],
                        x_slots[s].ap()[:, k * P : (k + 1) * P],
                        transpose=True,
                    )
                return xt, last

            wt = wt_pool.tile([P, KT, OUT_SH], BF16, tag="wt")

            def prep_otile(o):
                rows = slice(o * P, (o + 1) * P)
                wq_i = wq_pool.tile([P, IN], INT32, tag="wq")
                nc.sync.dma_start(wq_i[:], w_q.ap()[rows, :])
                wd = wd_pool.tile([P, IN], BF16, tag="wd")
                nc.vector.tensor_mul(
                    wd[:].rearrange("p (b q) -> p b q", q=QK),
                    wq_i[:].rearrange("p (b q) -> p b q", q=QK),
                    sc_tiles[o][:].unsqueeze(2).to_broadcast([P, NB, QK]),
                )
                for k in range(KT):
                    nc.sync.dma_start(
                        wt[:, k, o * P : (o + 1) * P],
                        wd[:, k * P : (k + 1) * P],
                        transpose=True,
                    )

            def gemm_group(xt, b, m, n):
                tok0 = b * TOK_BLK + m * P
                ps = psum_pool.tile([P, NCOL], FP32, tag="ps")
                for k in range(KT):
                    nc.tensor.matmul(
                        ps[:],
                        xt[:, k, m * P : (m + 1) * P],
                        wt[:, k, n * NCOL : (n + 1) * NCOL],
                        start=(k == 0),
                        stop=(k == KT - 1),
                    )
                ob = out_pool.tile([P, NCOL], FP32, tag="ob")
                nc.vector.tensor_add(
                    ob[:], ps[:], bias_rep[:, n * NCOL : (n + 1) * NCOL]
                )
                nc.sync.dma_start(
                    out.ap()[tok0 : tok0 + P, n * NCOL : (n + 1) * NCOL], ob[:]
                )

            # --- pipeline head: w o-tiles for n=0, then phase-1 GEMMs ---
            xt_cache = {}
            xt_last = {}
            for o in range(OT_PER_N):
                prep_otile(o)
            for b in range(NB1):
                xt_cache[b], xt_last[b] = load_xt(b)
                for m in range(MT):
                    gemm_group(xt_cache[b], b, m, 0)
            for o in range(OT_PER_N, OT):
                prep_otile(o)

            # --- main loop ---
            ncast = min(FREE_CASTS, NBLK)
            for b in range(NBLK):
                if b in xt_cache:
                    xt = xt_cache.pop(b)
                else:
                    xt, xt_last[b] = load_xt(b)
                for m in range(MT):
                    for n in range(NT):
                        if b < NB1 and n == 0:
                            continue
                        gemm_group(xt, b, m, n)
                if ncast < NBLK:
                    ci = cast_block(ncast)
                    add_dep_helper(
                        ci.ins, xt_last[b].ins, reason="pace x casts behind xt loads"
                    )
                    ncast += 1
    return nc


_COMPILED_NC = None


def _get_nc():
    global _COMPILED_NC
    if _COMPILED_NC is None:
        nc = bacc.Bacc("TRN2", target_bir_lowering=False, debug=False)
        build_kernel(nc)
        nc.compile()
        _COMPILED_NC = nc
    return _COMPILED_NC


def kernel(x, w_q, w_scales, bias):
    assert x.shape == (TOK, IN) and w_q.shape == (OUT, IN)
    nc = _get_nc()
    x = np.ascontiguousarray(np.asarray(x, dtype=np.float32))
    w_q = np.asarray(w_q, dtype=np.int32)
    w_scales = np.asarray(w_scales, dtype=np.float32)
    bias = np.asarray(bias, dtype=np.float32)
    in_maps = []
    for c in range(N_CORES):
        r = slice(c * OUT_SH, (c + 1) * OUT_SH)
        in_maps.append(
            {
                "x": x,
                "w_q": np.ascontiguousarray(w_q[r]),
                "w_scales": np.ascontiguousarray(w_scales[r]),
                "bias": np.ascontiguousarray(bias[r]),
            }
        )
    res = run_bass_kernel_spmd(nc, in_maps, list(range(N_CORES)))
    return np.concatenate([res.results[c]["out"] for c in range(N_CORES)], axis=1)


# revision 4
# speedup vs baseline: 2.0257x; 2.0257x over previous
"""Trainium2 Bass kernel for DequantingLinear (GGML Q8_0 dequant + linear).

Computes out[4096, 12288] = x[4096, 3072] @ dequant(w_q, w_scales).T + bias
where w_q is int32 (int8-valued) with per-32-element-block fp32 scales.

Sharding: tensor-parallel over output features across 8 NeuronCores. Each
core gets the full x and a 1536-row shard of w_q / w_scales / bias,
computes its [4096, 1536] output slice; the host concatenates on axis 1.

Per-core kernel (Tile framework):
  * w path: per 128-row o-tile, load w_q int32 -> SBUF, one mixed-dtype
    vector multiply (int32 x block-broadcast fp32 scales -> bf16, exact for
    |q|<=127), then ONE SBUF->SBUF xbar-transpose DMA ([128,3072] ->
    [128,24,128]) into the resident [in, k, out] SBUF weight tensor. No
    DRAM bounce for weights; 12 transpose instructions total.
  * x path: fp32 -> bf16 DRAM->DRAM SWDGE casts into a 4-slot ring of
    internal DRAM tensors, then ONE xbar-transpose per 512-token block
    ([512,3072] -> [128,24,512]). Casts beyond the first two are paced by
    a dependency on the xt transpose two blocks back.
  * GEMM: per (block, m) the three n-column psum tiles accumulate with the
    n-loop inside the k-loop, so consecutive matmuls share the same
    stationary operand (one LDWEIGHTS per k instead of three). Drains merge
    into one [128,1536] SBUF tile (bias added on the vector engine) and a
    single DMA writes full output rows. Phase-1 GEMMs (first two token
    blocks on n=0, first block on n=1) are issued against partial w so the
    PE has work while the w stream lands.
  All HWDGE DMAs are issued on nc.sync - ACT-issued DMAs were observed to
  produce corrupted results on hardware in this configuration.
"""

import sys

for _p in ("/opt/trn_rl_repo",):
    if _p not in sys.path:
        sys.path.append(_p)

from contextlib import ExitStack

import numpy as np

import concourse.bacc as bacc
import concourse.bass as bass
import concourse.mybir as mybir
from concourse import tile
from concourse.tile_rust import add_dep_helper
from concourse.bass_utils import run_bass_kernel_spmd

FP32 = mybir.dt.float32
BF16 = mybir.dt.bfloat16
INT32 = mybir.dt.int32

N_CORES = 8
TOK, IN, OUT = 4096, 3072, 12288
QK = 32
OUT_SH = OUT // N_CORES
TOK_BLK = 512
NCOL = 512
RING_SLOTS = 4
FREE_CASTS = 2
NB1 = 2


def build_kernel(nc: bass.Bass, repeats: int = 1):
    P = 128
    KT = IN // P          # 24 k-tiles
    NBLK = TOK // TOK_BLK  # 8 token blocks
    MT = TOK_BLK // P     # 4 m-tiles per block
    NT = OUT_SH // NCOL   # 3 n-column groups
    NB = IN // QK         # 96 scale blocks
    OT = OUT_SH // P      # 12 o-tiles
    OT_PER_N = NCOL // P  # 4 o-tiles per n-group

    x = nc.dram_tensor("x", [TOK, IN], FP32, kind="ExternalInput")
    w_q = nc.dram_tensor("w_q", [OUT_SH, IN], INT32, kind="ExternalInput")
    w_scales = nc.dram_tensor("w_scales", [OUT_SH, NB], FP32, kind="ExternalInput")
    bias = nc.dram_tensor("bias", [OUT_SH], FP32, kind="ExternalInput")
    out = nc.dram_tensor("out", [TOK, OUT_SH], FP32, kind="ExternalOutput")

    x_slots = [
        nc.dram_tensor(f"x_bf16_{s}", [TOK_BLK, IN], BF16) for s in range(RING_SLOTS)
    ]

    with tile.TileContext(nc) as tc, ExitStack() as ctx:
        const_pool = ctx.enter_context(tc.tile_pool(name="const", bufs=1))
        wq_pool = ctx.enter_context(tc.tile_pool(name="wq", bufs=2))
        wd_pool = ctx.enter_context(tc.tile_pool(name="wd", bufs=2))
        wt_pool = ctx.enter_context(tc.tile_pool(name="wt", bufs=1))
        xt_pool = ctx.enter_context(tc.tile_pool(name="xt", bufs=2))
        out_pool = ctx.enter_context(tc.tile_pool(name="out", bufs=3))
        ob1_pool = ctx.enter_context(tc.tile_pool(name="ob1", bufs=3))
        psum_pool = ctx.enter_context(tc.tile_pool(name="psum", bufs=8, space="PSUM"))

        for _rep in range(repeats):
            bias_rep = const_pool.tile([P, OUT_SH], FP32, tag="bias_rep")
            nc.sync.dma_start(
                bias_rep[:], bias.ap().unsqueeze(0).to_broadcast([P, OUT_SH])
            )

            sc_tiles = []
            for o in range(OT):
                sct = const_pool.tile([P, NB], FP32, tag=f"sc_{o}")
                nc.sync.dma_start(sct[:], w_scales.ap()[o * P : (o + 1) * P, :])
                sc_tiles.append(sct)

            def cast_block(b):
                s = b % RING_SLOTS
                srows = slice(b * TOK_BLK, (b + 1) * TOK_BLK)
                return nc.gpsimd.dma_start(x_slots[s].ap()[:, :], x.ap()[srows, :])

            for b in range(min(FREE_CASTS, NBLK)):
                cast_block(b)

            def load_xt(b):
                s = b % RING_SLOTS
                xt = xt_pool.tile([P, KT, TOK_BLK], BF16, tag="xt")
                last = nc.sync.dma_start(
                    xt[:, :, :], x_slots[s].ap()[:, :], transpose=True
                )
                return xt, last

            wt = wt_pool.tile([P, KT, OUT_SH], BF16, tag="wt")

            def prep_otile(o):
                rows = slice(o * P, (o + 1) * P)
                wq_i = wq_pool.tile([P, IN], INT32, tag="wq")
                nc.sync.dma_start(wq_i[:], w_q.ap()[rows, :])
                wd = wd_pool.tile([P, IN], BF16, tag="wd")
                nc.vector.tensor_mul(
                    wd[:].rearrange("p (b q) -> p b q", q=QK),
                    wq_i[:].rearrange("p (b q) -> p b q", q=QK),
                    sc_tiles[o][:].unsqueeze(2).to_broadcast([P, NB, QK]),
                )
                nc.sync.dma_start(
                    wt[:, :, o * P : (o + 1) * P], wd[:, :], transpose=True
                )

            def gemm_multi(xt, b, m, ns):
                """Accumulate len(ns) psum tiles with the n-loop inside the
                k-loop so consecutive matmuls share the stationary operand."""
                tok0 = b * TOK_BLK + m * P
                pss = [
                    psum_pool.tile([P, NCOL], FP32, tag="ps", name="ps") for _ in ns
                ]
                for k in range(KT):
                    for i, n in enumerate(ns):
                        nc.tensor.matmul(
                            pss[i][:],
                            xt[:, k, m * P : (m + 1) * P],
                            wt[:, k, n * NCOL : (n + 1) * NCOL],
                            start=(k == 0),
                            stop=(k == KT - 1),
                        )
                if len(ns) == NT:
                    ob = out_pool.tile([P, OUT_SH], FP32, tag="ob")
                    for i, n in enumerate(ns):
                        nc.vector.tensor_add(
                            ob[:, n * NCOL : (n + 1) * NCOL],
                            pss[i][:],
                            bias_rep[:, n * NCOL : (n + 1) * NCOL],
                        )
                    nc.sync.dma_start(out.ap()[tok0 : tok0 + P, :], ob[:])
                else:
                    for i, n in enumerate(ns):
                        ob = ob1_pool.tile([P, NCOL], FP32, tag="ob1")
                        nc.vector.tensor_add(
                            ob[:], pss[i][:], bias_rep[:, n * NCOL : (n + 1) * NCOL]
                        )
                        nc.sync.dma_start(
                            out.ap()[tok0 : tok0 + P, n * NCOL : (n + 1) * NCOL],
                            ob[:],
                        )

            # --- pipeline head: w o-tiles for n=0, then phase-1 GEMMs ---
            # phase1[b] = set of n already computed during the head
            phase1 = {b: set() for b in range(NBLK)}
            xt_cache = {}
            xt_last = {}
            for o in range(OT_PER_N):
                prep_otile(o)
            for b in range(NB1):
                xt_cache[b], xt_last[b] = load_xt(b)
                for m in range(MT):
                    gemm_multi(xt_cache[b], b, m, [0])
                phase1[b].add(0)
            for o in range(OT_PER_N, 2 * OT_PER_N):
                prep_otile(o)
            for m in range(MT):
                gemm_multi(xt_cache[0], 0, m, [1])
            phase1[0].add(1)
            for o in range(2 * OT_PER_N, OT):
                prep_otile(o)

            # --- main loop ---
            ncast = min(FREE_CASTS, NBLK)
            for b in range(NBLK):
                if b in xt_cache:
                    xt = xt_cache.pop(b)
                else:
                    xt, xt_last[b] = load_xt(b)
                ns = [n for n in range(NT) if n not in phase1[b]]
                for m in range(MT):
                    gemm_multi(xt, b, m, ns)
                if ncast < NBLK:
                    ci = cast_block(ncast)
                    add_dep_helper(
                        ci.ins, xt_last[b].ins, reason="pace x casts behind xt loads"
                    )
                    ncast += 1
    return nc


_COMPILED_NC = None


def _get_nc():
    global _COMPILED_NC
    if _COMPILED_NC is None:
        nc = bacc.Bacc("TRN2", target_bir_lowering=False, debug=False)
        build_kernel(nc)
        nc.compile()
        _COMPILED_NC = nc
    return _COMPILED_NC


def kernel(x, w_q, w_scales, bias):
    assert x.shape == (TOK, IN) and w_q.shape == (OUT, IN)
    nc = _get_nc()
    x = np.ascontiguousarray(np.asarray(x, dtype=np.float32))
    w_q = np.asarray(w_q, dtype=np.int32)
    w_scales = np.asarray(w_scales, dtype=np.float32)
    bias = np.asarray(bias, dtype=np.float32)
    in_maps = []
    for c in range(N_CORES):
        r = slice(c * OUT_SH, (c + 1) * OUT_SH)
        in_maps.append(
            {
                "x": x,
                "w_q": np.ascontiguousarray(w_q[r]),
                "w_scales": np.ascontiguousarray(w_scales[r]),
                "bias": np.ascontiguousarray(bias[r]),
            }
        )
    res = run_bass_kernel_spmd(nc, in_maps, list(range(N_CORES)))
    return np.concatenate([res.results[c]["out"] for c in range(N_CORES)], axis=1)
